# revision 32
# baseline (speedup 1.0000x reference)
"""Trainium2 Bass kernel for batched box-constrained QP projection.

Per sample s (B=8192 total, data-parallel over 8 cores):
    min_x 0.5||x - x_raw||^2 + p*||relu(A x - b)||^2,  0 <= x <= 100

The objective is 1-strongly convex with L = 1 + 2p*sigma_max(A)^2 (~9), so
instead of the reference's 200 plain-FISTA iterations we run Nesterov's
strongly-convex accelerated projected gradient with per-sample constant
momentum beta = (sqrt(L)-1)/(sqrt(L)+1): linear convergence, 8 iterations
reach rel err ~5e-3 vs the reference (gate is 2e-2). sigma_max^2 comes from
one unnormalized power iteration via the norm-ratio estimator
sqrt(||A^TA v0||/||v0||) (estimate accuracy only perturbs the step size).

Per-core layout (1024 samples, 8 blocks of 128 = 2 halves of 64):
  - matvecs z=A y / w=A^T r run on the PE via per-sample "diagonal
    stationary" blocks in fp16 (1 cycle/row vs fp32's 4): lhsT is a [K,32]
    fp16 block that is all zeros except column (p mod 32) holding the
    sample's vector; with tile_position=(0,32*(p//32)) the result lands in
    psum row p (fp32 accumulate). 64 matmuls accumulate a [64,85] z tile.
  - all pointwise math runs batched fp32 on [64, N] tiles (DVE/ACT),
  - per half-iteration a PE transpose + one strided DVE scatter (with
    fp32->fp16 cast) rebuilds the diagonal stationaries from y / r.
  - the two halves are emitted separately (separate diag tiles and psum
    banks) so half B's matmuls overlap half A's pointwise chain on DVE/ACT.
"""
import dataclasses
import math
from contextlib import ExitStack

import numpy as np

import concourse.bass as bass
import concourse.tile as tile
from concourse import mybir
from concourse.bass import ds
from concourse.bass_utils import run_bass_kernel_spmd
from concourse.masks import make_identity

# problem constants (hardcoded per spec)
B_TOTAL = 8192
N_CORES = 8
B_CORE = B_TOTAL // N_CORES       # 1024
BLK = 128                          # samples per block
H = 64                             # samples per half
NBLK = B_CORE // BLK               # 8
N = 80                             # x dim
M = 85                             # constraint dim
P_SLACK = 1.0
ITERS = 8                          # accelerated-gradient iterations (even)
UNROLL = 4
PITERS = 1                         # power iterations
F32 = mybir.dt.float32
F16 = mybir.dt.float16


def _diag_dest(region_ap):
    """Scatter destination: for local sample p (0..63), block p occupies
    cols [32p, 32p+32); the vector goes to column offset (p mod 32).
    col = 1024*(p//32) + 33*(p%32)."""
    pstride, pcount = region_ap.ap[0]
    return dataclasses.replace(
        region_ap,
        ap=[[pstride, pcount], [1024, 2], [33, 32]],
    )


def _emit_matvec_half(nc, bank, diag_region, mov_buf, mov_cols, h, skip=True,
                      preloaded=False):
    """64 matmuls for half h: psum row 64*h+p <- <diag block p> @ mov slice.
    Col-groups 2h/2h+1 alternate per instruction so each implicit LDWEIGHTS
    overlaps the other group's in-flight MM.  With preloaded=True the psum
    region holds a bias written beforehand and every matmul accumulates."""
    for o in range(32):
        for cl in range(2):
            c = 2 * h + cl
            blk_i = 32 * c + o          # sample index within the 128-block
            lb = blk_i - 64 * h         # local sample within the half
            nc.tensor.matmul(
                bank[32 * c:32 * c + 32, 0:mov_cols],
                diag_region[:, 32 * lb:32 * lb + 32],
                mov_buf[:, mov_cols * blk_i:mov_cols * blk_i + mov_cols],
                start=(o == 0) and not preloaded, stop=(o == 31),
                tile_position=(0, 32 * c),
                skip_group_check=skip,
            )


def _split_multiwait_insts(nc):
    """walrus codegen allows only ONE sync-wait on compute/Drain instructions
    (setupSyncWait: 'Too many sync wait commands').  Tile can emit several.
    Peel all-but-one wait off onto same-engine single-wait NoOps placed just
    before the instruction (same engine + program order => identical
    semantics).  Barrier NoOps are left untouched."""
    cnt = 0
    for f in nc.m.functions:
        for b in f.blocks:
            il = list(b.instructions)
            out = []
            changed = False
            for ins in il:
                si = getattr(ins, "sync_info", None)
                if (
                    si is not None
                    and len(si.on_wait) > 1
                    and ins.opcode != "ISA"
                ):
                    waits = list(si.on_wait)
                    for j, w in enumerate(waits[:-1]):
                        nd = mybir.InstDrain(
                            name=f"{ins.name}-sw{j}", engine=ins.engine,
                            ins=[], outs=[],
                        )
                        nd.sync_info = mybir.SyncInfo(on_wait=[w], on_update=[])
                        out.append(nd)
                        cnt += 1
                    ins.sync_info = mybir.SyncInfo(
                        on_wait=[waits[-1]], on_update=list(si.on_update)
                    )
                    changed = True
                out.append(ins)
            if changed:
                b.instructions = out
    return cnt


def build_kernel(nc, split_waits=True, iters=None, piters=None, null_body=False,
                 repeat=1, no_adma=False):
    iters = ITERS if iters is None else iters
    piters = PITERS if piters is None else piters
    x_raw_d = nc.dram_tensor("x_raw", [B_CORE, N], F32, kind="ExternalInput").ap()
    A_d = nc.dram_tensor("Ap", [NBLK, M, BLK * N], F16, kind="ExternalInput").ap()
    AT_d = nc.dram_tensor("ATp", [NBLK, N, BLK * M], F16, kind="ExternalInput").ap()
    b_d = nc.dram_tensor("b", [B_CORE, M], F32, kind="ExternalInput").ap()
    out_d = nc.dram_tensor("x_out", [B_CORE, N], F32, kind="ExternalOutput").ap()

    if null_body:
        # calibration build: same external I/O, near-zero device work
        with tile.TileContext(nc) as tc, ExitStack() as ctx:
            state = ctx.enter_context(tc.tile_pool(name="state", bufs=1))
            xraw_t = state.tile([BLK, N], F32)
            with tc.For_i(0, NBLK, 1, name="blk") as bi:
                nc.sync.dma_start(xraw_t[:], x_raw_d[ds(bi * BLK, BLK), :])
                nc.vector.tensor_scalar(
                    xraw_t[:], xraw_t[:], 0.0, 100.0,
                    op0=mybir.AluOpType.max, op1=mybir.AluOpType.min,
                )
                nc.sync.dma_start(out_d[ds(bi * BLK, BLK), :], xraw_t[:])
        if split_waits:
            _split_multiwait_insts(nc)
        return nc

    with tile.TileContext(nc) as tc, ExitStack() as ctx:
        consts = ctx.enter_context(tc.tile_pool(name="consts", bufs=1))
        abuf = ctx.enter_context(tc.tile_pool(name="abuf", bufs=1))
        state = ctx.enter_context(tc.tile_pool(name="state", bufs=1))
        ps = ctx.enter_context(tc.tile_pool(name="ps", bufs=1, space="PSUM"))

        ident = consts.tile([128, 128], F32)
        make_identity(nc, ident)

        # per-half diagonal stationary regions, fp16 (off-diagonal zeros
        # persist forever)
        y_diag = [consts.tile([N, 32 * H], F16, name=f"ydiag{h}") for h in range(2)]
        r_diag = [consts.tile([M, 32 * H], F16, name=f"rdiag{h}") for h in range(2)]
        for h in range(2):
            nc.vector.memset(y_diag[h][:], 0.0)
            nc.vector.memset(r_diag[h][:], 0.0)

        # per-block A buffers, fp16 (sample-major along free dim); two sets
        # so block bi+1's DMA overlaps block bi's compute
        AT_buf = [abuf.tile([N, BLK * M], F16, name=f"ATb{s}") for s in range(2)]
        A_buf = [abuf.tile([M, BLK * N], F16, name=f"Ab{s}") for s in range(2)]

        # per-half state tiles: halves of [128, x] parents so that every
        # SB operand of a half shares the same base partition (64*h)
        def half_tiles(name, cols):
            t = state.tile([BLK, cols], F32, name=name)
            return t, [t[H * hh:H * hh + H, :] for hh in range(2)]
        y_t, y_sb = half_tiles("y_t", N)
        xa_t, xa = half_tiles("xa_t", N)
        xb_t, xb = half_tiles("xb_t", N)
        xraw_t, xraw_sb = half_tiles("xraw_t", N)
        b_t, b_sb = half_tiles("b_t", M)
        r_t, r_sb = half_tiles("r_t", M)
        g_t, g_sb = half_tiles("g_t", N)
        u_t, u_sb = half_tiles("u_t", N)
        mb_t, mb_sb = half_tiles("mb_t", N)
        av_t, av_sb = half_tiles("av_t", M)
        # scalars: 0 nrm1, 1 nrm2, 2 tmp/sig2, 3 ratio/sqL, 4 L,
        #          5 step, 6 negstep, 7 beta, 8 negbeta, 9 1+beta,
        #          10 -2p*step
        sc_t, sc_sb = half_tiles("sc_t", 11)
        negbeta_c = lambda hh: sc_sb[hh][:, 8:9]
        onepb_c = lambda hh: sc_sb[hh][:, 9:10]
        n2ps_c = lambda hh: sc_sb[hh][:, 10:11]
        # x_raw/(2p), for the w-psum gradient preload
        xrs_t, xrs_sb = half_tiles("xrs_t", N)

        # psum tiles (one bank each); half h occupies rows [64h, 64h+64)
        z_ps_t = [ps.tile([128, 512], F32, name=f"z{h}") for h in range(2)]
        w_ps_t = [ps.tile([128, 512], F32, name=f"w{h}") for h in range(2)]
        t1_ps_t = [ps.tile([128, 512], F32, name=f"t1{h}") for h in range(2)]
        t2_ps_t = [ps.tile([128, 512], F32, name=f"t2{h}") for h in range(2)]
        z_ps = [z_ps_t[hh][H * hh:H * hh + H, 0:M] for hh in range(2)]
        w_ps = [w_ps_t[hh][H * hh:H * hh + H, 0:N] for hh in range(2)]

        def scatter(dst_region, src_T):
            # src_T: psum [dim, 64] fp32; dst: fp16 diag blocks (cast on copy).
            # Runs on ACT: the next matvec blocks on this copy, and the DVE
            # always has a ready pointwise op that would delay it ~200ns.
            with tc.high_priority():
                nc.scalar.copy(
                    _diag_dest(dst_region),
                    src_T.rearrange("x (c o) -> x c o", o=32),
                )

        def transpose_scatter(vec_sb, dst_region, t_tile, half, dim):
            tp = t_tile[0:dim, 0:H]
            idh = ident[H * half:H * half + H, H * half:H * half + H]
            nc.tensor.transpose(tp, vec_sb[:, 0:dim], idh)
            scatter(dst_region, tp)

        if no_adma:
            # timing probe: load blocks 0/1 once, skip per-block A DMAs
            for s in range(2):
                nc.sync.dma_start(AT_buf[s][:], AT_d[ds(s, 1), :, :].rearrange("o n x -> (o n) x"))
                nc.sync.dma_start(A_buf[s][:], A_d[ds(s, 1), :, :].rearrange("o m x -> (o m) x"))
        rep_ctx = tc.For_i(0, repeat, 1, name="rep") if repeat > 1 else None
        if rep_ctx is not None:
            rep_ctx.__enter__()
        with tc.For_i(0, NBLK, 2, name="blk") as bi2_:
            # timing builds (repeat>1) pin DRAM addresses to block 0 so no
            # symbolic DMA APs are needed under the nested loop (SP register
            # pressure); sizes and traffic are identical.
            bi2 = 0 if repeat > 1 else bi2_
            if not no_adma:
                for s in range(2):
                    nc.sync.dma_start(
                        AT_buf[s][:],
                        AT_d[ds(bi2 + s, 1), :, :].rearrange("o n x -> (o n) x"))
                    nc.sync.dma_start(
                        A_buf[s][:],
                        A_d[ds(bi2 + s, 1), :, :].rearrange("o m x -> (o m) x"))
            for s in range(2):
                AT_b, A_b = AT_buf[s], A_buf[s]
                for h in range(2):
                    # small input loads go on the ACT hardware DMA queue so
                    # their waits never head-of-line-block the SP queue
                    # that streams the big A prefetches
                    nc.scalar.dma_start(
                        xraw_sb[h][:], x_raw_d[ds(bi2 * BLK + (s * BLK + H * h), H), :])
                    nc.scalar.dma_start(
                        b_sb[h][:], b_d[ds(bi2 * BLK + (s * BLK + H * h), H), :])

                # x0 = clip(x_raw) early: xb/y are free during the power
                # phase, and this keeps the post-power critical path short
                for h in range(2):
                    nc.vector.tensor_scalar(
                        xb[h], xraw_sb[h][:], 0.0, 100.0,
                        op0=mybir.AluOpType.max, op1=mybir.AluOpType.min,
                    )
                    nc.vector.tensor_copy(y_sb[h][:], xb[h])

                # ---- power iteration: v <- A^T A v (unnormalized; values
                #      stay O(20) so fp16 is safe).  v0 = ones is memset
                #      straight into the diagonal slots; nrm_k = ||v_k||^2
                #      accumulates off the critical path. ----
                for pi in range(piters):
                    for h in range(2):
                        if pi == 0:
                            nc.vector.memset(_diag_dest(y_diag[h][:]), 1.0)
                        else:
                            transpose_scatter(u_sb[h], y_diag[h][:],
                                              t1_ps_t[h], h, N)
                        _emit_matvec_half(nc, z_ps_t[h][:], y_diag[h][:], AT_b, M, h)
                    for h in range(2):
                        nc.vector.tensor_copy(av_sb[h][:], z_ps[h])
                        transpose_scatter(av_sb[h], r_diag[h][:], t2_ps_t[h], h, M)
                        _emit_matvec_half(nc, w_ps_t[h][:], r_diag[h][:], A_b, N, h)
                    for h in range(2):
                        nrm = sc_sb[h][:, pi:pi + 1]
                        nc.vector.tensor_copy(u_sb[h][:], w_ps[h])
                        nc.vector.tensor_mul(g_sb[h][:], u_sb[h][:], u_sb[h][:])
                        nc.vector.reduce_sum(nrm, g_sb[h][:],
                                             axis=mybir.AxisListType.X)

                # ---- x0 scattered (critical path: last power matvec ->
                #      transpose -> scatter -> first z); the scalar chain
                #      sigma^2 = sqrt(nrm_k/nrm_{k-1}), L = 1+2p*sigma^2,
                #      step = 1/L, beta = (sqrt(L)-1)/(sqrt(L)+1)
                #      overlaps the first FISTA matvecs on DVE/ACT. ----
                for h in range(2):
                    transpose_scatter(xb[h], y_diag[h][:], t1_ps_t[h], h, N)
                # prologue z for k=0, accumulating onto the -b preload
                for h in range(2):
                    nc.vector.tensor_scalar_mul(z_ps[h], b_sb[h][:], -1.0)
                for h in range(2):
                    _emit_matvec_half(nc, z_ps_t[h][:], y_diag[h][:], AT_b, M, h,
                                      preloaded=True)
                for h in range(2):
                    nrm1 = sc_sb[h][:, 0:1]
                    nrm2 = sc_sb[h][:, 1:2]
                    tmp = sc_sb[h][:, 2:3]
                    ratio = sc_sb[h][:, 3:4]
                    L = sc_sb[h][:, 4:5]
                    step = sc_sb[h][:, 5:6]
                    negstep = sc_sb[h][:, 6:7]
                    beta = sc_sb[h][:, 7:8]
                    sig2 = tmp   # reuse after rinv consumed
                    sqL = ratio  # reuse after sig2 consumed
                    if piters == 1:
                        # ||v0||^2 = N exactly
                        nc.vector.tensor_scalar_mul(ratio, nrm1, 1.0 / N)
                    else:
                        nprev = sc_sb[h][:, piters - 2:piters - 1]
                        ncur = sc_sb[h][:, piters - 1:piters]
                        nc.vector.reciprocal(tmp, nprev)
                        nc.vector.tensor_mul(ratio, ncur, tmp)
                    nc.scalar.sqrt(sig2, ratio)
                    nc.vector.tensor_scalar(
                        L, sig2, 2.0 * P_SLACK, 1.0,
                        op0=mybir.AluOpType.mult, op1=mybir.AluOpType.add,
                    )
                    nc.vector.reciprocal(step, L)
                    nc.vector.tensor_scalar_mul(negstep, step, -1.0)
                    nc.scalar.sqrt(sqL, L)
                    nc.vector.tensor_scalar_add(nrm1, sqL, 1.0)
                    nc.vector.reciprocal(nrm2, nrm1)
                    nc.vector.scalar_tensor_tensor(
                        beta, sqL, -1.0, nrm2,
                        op0=mybir.AluOpType.add, op1=mybir.AluOpType.mult,
                    )
                    nc.vector.tensor_scalar_mul(negbeta_c(h), beta, -1.0)
                    nc.vector.tensor_scalar_add(onepb_c(h), beta, 1.0)
                    nc.vector.tensor_scalar_mul(n2ps_c(h), step, -2.0 * P_SLACK)
                    nc.vector.tensor_scalar_mul(
                        xrs_sb[h][:], xraw_sb[h][:], 1.0 / (2.0 * P_SLACK))

                # ---- accelerated-gradient iterations (fully unrolled,
                #      software-pipelined: iteration k's z matvecs were
                #      emitted at the tail of iteration k-1, so the PE order
                #      is tr_r0 tr_r1 w0 w1 tr_y0 tr_y1 z0' z1' with the DVE
                #      update chains hidden under the matvecs.
                #      The z psum is preloaded with -b (so relu(psum) = r in
                #      one ACT op) and the w psum with (y - x_raw)/2p (so
                #      x_new = clip(y - 2p*step*psum) in one DVE op); the
                #      matvecs accumulate onto the preloads (start=False).
                #      The last iteration skips the dead y-update/scatter. ----
                for k in range(iters):
                    last = (k == iters - 1)
                    # early DVE work for this k (overlaps the in-flight z):
                    # w-psum gradient preload; mb = -beta * x_old
                    for h in range(2):
                        x_old = xb[h] if k % 2 == 0 else xa[h]
                        nc.vector.scalar_tensor_tensor(
                            w_ps[h], y_sb[h][:], 1.0 / (2.0 * P_SLACK),
                            xrs_sb[h][:],
                            op0=mybir.AluOpType.mult,
                            op1=mybir.AluOpType.subtract,
                        )
                        if not last:
                            nc.vector.tensor_scalar_mul(
                                mb_sb[h][:], x_old, negbeta_c(h))
                    # r = relu(z-psum) (already z - b) -> scatter -> w = A^T r
                    for h in range(2):
                        nc.scalar.activation(r_sb[h][:], z_ps[h],
                                             mybir.ActivationFunctionType.Relu)
                        transpose_scatter(r_sb[h], r_diag[h][:], t2_ps_t[h], h, M)
                    for h in range(2):
                        _emit_matvec_half(nc, w_ps_t[h][:], r_diag[h][:], A_b, N, h,
                                          preloaded=True)
                    # pointwise update: x_new = clip(y + n2ps*w~) then
                    # y = (1+beta)*x_new + mb
                    for h in range(2):
                        x_new = xa[h] if k % 2 == 0 else xb[h]
                        nc.vector.scalar_tensor_tensor(
                            x_new, w_ps[h], n2ps_c(h), y_sb[h][:],
                            op0=mybir.AluOpType.mult, op1=mybir.AluOpType.add,
                        )
                        nc.vector.tensor_scalar(
                            x_new, x_new, 0.0, 100.0,
                            op0=mybir.AluOpType.max, op1=mybir.AluOpType.min,
                        )
                        if not last:
                            nc.vector.scalar_tensor_tensor(
                                y_sb[h][:], x_new, onepb_c(h), mb_sb[h][:],
                                op0=mybir.AluOpType.mult, op1=mybir.AluOpType.add,
                            )
                            transpose_scatter(y_sb[h], y_diag[h][:], t1_ps_t[h], h, N)
                    if not last:
                        # z-psum preload (-b) then z matvecs for k+1
                        for h in range(2):
                            nc.vector.tensor_scalar_mul(z_ps[h], b_sb[h][:], -1.0)
                        for h in range(2):
                            _emit_matvec_half(nc, z_ps_t[h][:], y_diag[h][:],
                                              AT_b, M, h, preloaded=True)

                # final x is in the tile written by iteration iters-1
                # (iters even -> xb)
                assert iters % 2 == 0
                nc.sync.dma_start(out_d[ds(bi2 * BLK + s * BLK, BLK), :], xb_t[:])
        if rep_ctx is not None:
            rep_ctx.__exit__(None, None, None)

    if split_waits:
        _split_multiwait_insts(nc)
    return nc


_CACHED = {}


def _get_nc(**kw):
    key = ("nc",) + tuple(sorted(kw.items()))
    if key not in _CACHED:
        nc = bass.Bass("TRN2", target_bir_lowering=False, debug=False)
        build_kernel(nc, **kw)
        nc.finalize()
        _CACHED[key] = nc
    return _CACHED[key]


def _concat_in_maps(x_raw, A, b):
    per_core = []
    for c in range(N_CORES):
        sl = slice(c * B_CORE, (c + 1) * B_CORE)
        Ac = A[sl].reshape(NBLK, BLK, M, N)
        Ap = np.ascontiguousarray(
            Ac.transpose(0, 2, 1, 3)).reshape(NBLK, M, BLK * N).astype(np.float16)
        ATp = np.ascontiguousarray(
            Ac.transpose(0, 3, 1, 2)).reshape(NBLK, N, BLK * M).astype(np.float16)
        per_core.append({
            "x_raw": x_raw[sl], "Ap": Ap, "ATp": ATp, "b": b[sl],
        })
    return per_core


def timed_runs(inputs, n=5, nc=None):
    """Warm, device-resident-input executions; returns per-call wall ns."""
    import time
    import jax
    from jax.sharding import Mesh, PartitionSpec, NamedSharding
    from jax.experimental.shard_map import shard_map
    from concourse import bass2jax

    bass2jax.install_neuronx_cc_hook()
    if nc is None:
        nc = _get_nc()
    x_raw = np.ascontiguousarray(inputs["x_raw"], np.float32)
    A = np.ascontiguousarray(inputs["A"], np.float32)
    b = np.ascontiguousarray(inputs["b"], np.float32)
    per_core = _concat_in_maps(x_raw, A, b)

    in_names, out_names, out_avals = [], [], []
    for alloc in nc.m.functions[0].allocations:
        if not isinstance(alloc, mybir.MemoryLocationSet):
            continue
        name = alloc.memorylocations[0].name
        if alloc.kind == "ExternalInput":
            in_names.append(name)
        elif alloc.kind == "ExternalOutput":
            out_names.append(name)
            out_avals.append(jax.core.ShapedArray(
                tuple(alloc.tensor_shape), mybir.dt.np(alloc.dtype)))
    pid_name = nc.partition_id_tensor.name if nc.partition_id_tensor else None
    if pid_name is not None and pid_name in in_names:
        in_names.remove(pid_name)

    all_names = in_names + out_names
    if pid_name is not None:
        all_names = all_names + [pid_name]

    def _body(*args):
        operands = list(args)
        if pid_name is not None:
            operands.append(bass2jax.partition_id_tensor())
        outs = bass2jax._bass_exec_p.bind(
            *operands,
            out_avals=tuple(out_avals),
            in_names=tuple(all_names),
            out_names=tuple(out_names),
            lowering_input_output_aliases=(),
            sim_require_finite=True,
            sim_require_nnan=True,
            nc=nc,
        )
        return tuple(outs)

    devices = jax.devices()[:N_CORES]
    mesh = Mesh(np.asarray(devices), ("core",))
    nin = len(in_names) + len(out_names)
    fn = jax.jit(
        shard_map(_body, mesh=mesh, in_specs=(PartitionSpec("core"),) * nin,
                  out_specs=(PartitionSpec("core"),) * len(out_names),
                  check_rep=False),
        keep_unused=True,
    )
    sh = NamedSharding(mesh, PartitionSpec("core"))
    concat = [np.concatenate([pc[nm] for pc in per_core], axis=0) for nm in in_names]
    zeros = [np.zeros((N_CORES * av.shape[0], *av.shape[1:]), av.dtype)
             for av in out_avals]
    args = [jax.device_put(v, sh) for v in concat + zeros]
    out = fn(*args)
    jax.block_until_ready(out)  # compile + warmup
    times = []
    for _ in range(n):
        t0 = time.perf_counter()
        out = fn(*args)
        jax.block_until_ready(out)
        times.append((time.perf_counter() - t0) * 1e9)
    return times


def kernel(x_raw, A, b, lower, upper):
    x_raw = np.ascontiguousarray(x_raw, np.float32)
    A = np.ascontiguousarray(A, np.float32)
    b = np.ascontiguousarray(b, np.float32)

    nc = _get_nc()
    in_maps = _concat_in_maps(x_raw, A, b)
    res = run_bass_kernel_spmd(nc, in_maps, core_ids=list(range(N_CORES)))
    _CACHED["last_result"] = res
    out = np.concatenate([res.results[c]["x_out"] for c in range(N_CORES)], axis=0)
    return out.astype(np.float32)


# revision 33
# speedup vs baseline: 1.1701x; 1.1701x over previous
"""Trainium2 Bass kernel for batched box-constrained QP projection.

Per sample s (B=8192 total, data-parallel over 8 cores):
    min_x 0.5||x - x_raw||^2 + p*||relu(A x - b)||^2,  0 <= x <= 100

The objective is 1-strongly convex with L = 1 + 2p*sigma_max(A)^2 (~9), so
instead of the reference's 200 plain-FISTA iterations we run Nesterov's
strongly-convex accelerated projected gradient with per-sample constant
momentum beta = (sqrt(L)-1)/(sqrt(L)+1): linear convergence, 8 iterations
reach rel err ~5e-3 vs the reference (gate is 2e-2). sigma_max^2 comes from
one unnormalized power iteration via the norm-ratio estimator
sqrt(||A^TA v0||/||v0||) (estimate accuracy only perturbs the step size).

Per-core layout (1024 samples, 8 blocks of 128 = 2 halves of 64):
  - matvecs z=A y / w=A^T r run on the PE via per-sample "diagonal
    stationary" blocks in fp16 (1 cycle/row vs fp32's 4): lhsT is a [K,32]
    fp16 block that is all zeros except column (p mod 32) holding the
    sample's vector; with tile_position=(0,32*(p//32)) the result lands in
    psum row p (fp32 accumulate). 64 matmuls accumulate a [64,85] z tile.
  - all pointwise math runs batched fp32 on [64, N] tiles (DVE/ACT),
  - per half-iteration a PE transpose + one strided DVE scatter (with
    fp32->fp16 cast) rebuilds the diagonal stationaries from y / r.
  - the two halves are emitted separately (separate diag tiles and psum
    banks) so half B's matmuls overlap half A's pointwise chain on DVE/ACT.
"""
import dataclasses
import math
from contextlib import ExitStack

import numpy as np

import concourse.bass as bass
import concourse.tile as tile
from concourse import mybir
from concourse.bass import ds
from concourse.bass_utils import run_bass_kernel_spmd
from concourse.masks import make_identity

# problem constants (hardcoded per spec)
B_TOTAL = 8192
N_CORES = 8
B_CORE = B_TOTAL // N_CORES       # 1024
BLK = 128                          # samples per block
H = 64                             # samples per half
NBLK = B_CORE // BLK               # 8
N = 80                             # x dim
M = 85                             # constraint dim
P_SLACK = 1.0
ITERS = 8                          # accelerated-gradient iterations (even)
UNROLL = 4
PITERS = 1                         # power iterations
F32 = mybir.dt.float32
F16 = mybir.dt.float16


def _diag_dest(region_ap):
    """Scatter destination: for local sample p (0..63), block p occupies
    cols [32p, 32p+32); the vector goes to column offset (p mod 32).
    col = 1024*(p//32) + 33*(p%32)."""
    pstride, pcount = region_ap.ap[0]
    return dataclasses.replace(
        region_ap,
        ap=[[pstride, pcount], [1024, 2], [33, 32]],
    )


def _emit_matvec_half(nc, bank, diag_region, mov_buf, mov_cols, h, skip=True,
                      preloaded=False):
    """64 matmuls for half h: psum row 64*h+p <- <diag block p> @ mov slice.
    Col-groups 2h/2h+1 alternate per instruction so each implicit LDWEIGHTS
    overlaps the other group's in-flight MM.  With preloaded=True the psum
    region holds a bias written beforehand and every matmul accumulates."""
    for o in range(32):
        for cl in range(2):
            c = 2 * h + cl
            blk_i = 32 * c + o          # sample index within the 128-block
            lb = blk_i - 64 * h         # local sample within the half
            nc.tensor.matmul(
                bank[32 * c:32 * c + 32, 0:mov_cols],
                diag_region[:, 32 * lb:32 * lb + 32],
                mov_buf[:, mov_cols * blk_i:mov_cols * blk_i + mov_cols],
                start=(o == 0) and not preloaded, stop=(o == 31),
                tile_position=(0, 32 * c),
                skip_group_check=skip,
            )


def _split_multiwait_insts(nc):
    """walrus codegen allows only ONE sync-wait on compute/Drain instructions
    (setupSyncWait: 'Too many sync wait commands').  Tile can emit several.
    Peel all-but-one wait off onto same-engine single-wait NoOps placed just
    before the instruction (same engine + program order => identical
    semantics).  Barrier NoOps are left untouched."""
    cnt = 0
    for f in nc.m.functions:
        for b in f.blocks:
            il = list(b.instructions)
            out = []
            changed = False
            for ins in il:
                si = getattr(ins, "sync_info", None)
                if (
                    si is not None
                    and len(si.on_wait) > 1
                    and ins.opcode != "ISA"
                ):
                    waits = list(si.on_wait)
                    for j, w in enumerate(waits[:-1]):
                        nd = mybir.InstDrain(
                            name=f"{ins.name}-sw{j}", engine=ins.engine,
                            ins=[], outs=[],
                        )
                        nd.sync_info = mybir.SyncInfo(on_wait=[w], on_update=[])
                        out.append(nd)
                        cnt += 1
                    ins.sync_info = mybir.SyncInfo(
                        on_wait=[waits[-1]], on_update=list(si.on_update)
                    )
                    changed = True
                out.append(ins)
            if changed:
                b.instructions = out
    return cnt


def build_kernel(nc, split_waits=True, iters=None, piters=None, null_body=False,
                 repeat=1, no_adma=False):
    iters = ITERS if iters is None else iters
    piters = PITERS if piters is None else piters
    x_raw_d = nc.dram_tensor("x_raw", [B_CORE, N], F32, kind="ExternalInput").ap()
    A_d = nc.dram_tensor("Ap", [NBLK, M, BLK * N], F16, kind="ExternalInput").ap()
    AT_d = nc.dram_tensor("ATp", [NBLK, N, BLK * M], F16, kind="ExternalInput").ap()
    b_d = nc.dram_tensor("b", [B_CORE, M], F32, kind="ExternalInput").ap()
    out_d = nc.dram_tensor("x_out", [B_CORE, N], F32, kind="ExternalOutput").ap()

    if null_body:
        # calibration build: same external I/O, near-zero device work
        with tile.TileContext(nc) as tc, ExitStack() as ctx:
            state = ctx.enter_context(tc.tile_pool(name="state", bufs=1))
            xraw_t = state.tile([BLK, N], F32)
            with tc.For_i(0, NBLK, 1, name="blk") as bi:
                nc.sync.dma_start(xraw_t[:], x_raw_d[ds(bi * BLK, BLK), :])
                nc.vector.tensor_scalar(
                    xraw_t[:], xraw_t[:], 0.0, 100.0,
                    op0=mybir.AluOpType.max, op1=mybir.AluOpType.min,
                )
                nc.sync.dma_start(out_d[ds(bi * BLK, BLK), :], xraw_t[:])
        if split_waits:
            _split_multiwait_insts(nc)
        return nc

    with tile.TileContext(nc) as tc, ExitStack() as ctx:
        consts = ctx.enter_context(tc.tile_pool(name="consts", bufs=1))
        abuf = ctx.enter_context(tc.tile_pool(name="abuf", bufs=1))
        state = ctx.enter_context(tc.tile_pool(name="state", bufs=1))
        ps = ctx.enter_context(tc.tile_pool(name="ps", bufs=1, space="PSUM"))

        ident = consts.tile([128, 128], F32)
        make_identity(nc, ident)

        # per-half diagonal stationary regions, fp16 (off-diagonal zeros
        # persist forever)
        y_diag = [consts.tile([N, 32 * H], F16, name=f"ydiag{h}") for h in range(2)]
        r_diag = [consts.tile([M, 32 * H], F16, name=f"rdiag{h}") for h in range(2)]
        for h in range(2):
            nc.vector.memset(y_diag[h][:], 0.0)
            nc.vector.memset(r_diag[h][:], 0.0)

        # per-block A buffers, fp16 (sample-major along free dim); two sets
        # so block bi+1's DMA overlaps block bi's compute
        AT_buf = [abuf.tile([N, BLK * M], F16, name=f"ATb{s}") for s in range(2)]
        A_buf = [abuf.tile([M, BLK * N], F16, name=f"Ab{s}") for s in range(2)]

        # per-half state tiles: halves of [128, x] parents so that every
        # SB operand of a half shares the same base partition (64*h)
        def half_tiles(name, cols):
            t = state.tile([BLK, cols], F32, name=name)
            return t, [t[H * hh:H * hh + H, :] for hh in range(2)]
        y_t, y_sb = half_tiles("y_t", N)
        xa_t, xa = half_tiles("xa_t", N)
        xb_t, xb = half_tiles("xb_t", N)
        xraw_t, xraw_sb = half_tiles("xraw_t", N)
        b_t, b_sb = half_tiles("b_t", M)
        r_t, r_sb = half_tiles("r_t", M)
        g_t, g_sb = half_tiles("g_t", N)
        u_t, u_sb = half_tiles("u_t", N)
        mb_t, mb_sb = half_tiles("mb_t", N)
        av_t, av_sb = half_tiles("av_t", M)
        # scalars: 0 nrm1, 1 nrm2, 2 tmp/sig2, 3 ratio/sqL, 4 L,
        #          5 step, 6 negstep, 7 beta, 8 negbeta, 9 1+beta,
        #          10 -2p*step
        sc_t, sc_sb = half_tiles("sc_t", 11)
        negbeta_c = lambda hh: sc_sb[hh][:, 8:9]
        onepb_c = lambda hh: sc_sb[hh][:, 9:10]
        n2ps_c = lambda hh: sc_sb[hh][:, 10:11]
        # x_raw/(2p), for the w-psum gradient preload
        xrs_t, xrs_sb = half_tiles("xrs_t", N)

        # psum tiles (one bank each); half h occupies rows [64h, 64h+64)
        z_ps_t = [ps.tile([128, 512], F32, name=f"z{h}") for h in range(2)]
        w_ps_t = [ps.tile([128, 512], F32, name=f"w{h}") for h in range(2)]
        t1_ps_t = [ps.tile([128, 512], F32, name=f"t1{h}") for h in range(2)]
        t2_ps_t = [ps.tile([128, 512], F32, name=f"t2{h}") for h in range(2)]
        z_ps = [z_ps_t[hh][H * hh:H * hh + H, 0:M] for hh in range(2)]
        w_ps = [w_ps_t[hh][H * hh:H * hh + H, 0:N] for hh in range(2)]

        def scatter(dst_region, src_T):
            # src_T: psum [dim, 64] fp32; dst: fp16 diag blocks (cast on copy).
            # Runs on ACT: the next matvec blocks on this copy, and the DVE
            # always has a ready pointwise op that would delay it ~200ns.
            with tc.high_priority():
                nc.scalar.copy(
                    _diag_dest(dst_region),
                    src_T.rearrange("x (c o) -> x c o", o=32),
                )

        def transpose_scatter(vec_sb, dst_region, t_tile, half, dim):
            tp = t_tile[0:dim, 0:H]
            idh = ident[H * half:H * half + H, H * half:H * half + H]
            nc.tensor.transpose(tp, vec_sb[:, 0:dim], idh)
            scatter(dst_region, tp)

        if no_adma:
            # timing probe: load blocks 0/1 once, skip per-block A DMAs
            for s in range(2):
                nc.sync.dma_start(AT_buf[s][:], AT_d[ds(s, 1), :, :].rearrange("o n x -> (o n) x"))
                nc.sync.dma_start(A_buf[s][:], A_d[ds(s, 1), :, :].rearrange("o m x -> (o m) x"))
        rep_ctx = tc.For_i(0, repeat, 1, name="rep") if repeat > 1 else None
        if rep_ctx is not None:
            rep_ctx.__enter__()
        with tc.For_i(0, NBLK, 2, name="blk") as bi2_:
            # timing builds (repeat>1) pin DRAM addresses to block 0 so no
            # symbolic DMA APs are needed under the nested loop (SP register
            # pressure); sizes and traffic are identical.
            bi2 = 0 if repeat > 1 else bi2_
            if not no_adma:
                # set 0 is on the critical path at trip start (the enqueue
                # cannot cross the loop back-edge), so its loads are split
                # per half: the half-0 matvecs start after half the bytes
                for part in range(2):
                    nc.sync.dma_start(
                        AT_buf[0][:, part * (H * M):(part + 1) * (H * M)],
                        AT_d[ds(bi2, 1), :, part * (H * M):(part + 1) * (H * M)
                             ].rearrange("o n x -> (o n) x"))
                nc.sync.dma_start(
                    A_buf[0][:],
                    A_d[ds(bi2, 1), :, :].rearrange("o m x -> (o m) x"))
                nc.sync.dma_start(
                    AT_buf[1][:],
                    AT_d[ds(bi2 + 1, 1), :, :].rearrange("o n x -> (o n) x"))
                nc.sync.dma_start(
                    A_buf[1][:],
                    A_d[ds(bi2 + 1, 1), :, :].rearrange("o m x -> (o m) x"))
            for s in range(2):
                AT_b, A_b = AT_buf[s], A_buf[s]
                for h in range(2):
                    # small input loads go on the ACT hardware DMA queue so
                    # their waits never head-of-line-block the SP queue
                    # that streams the big A prefetches
                    nc.scalar.dma_start(
                        xraw_sb[h][:], x_raw_d[ds(bi2 * BLK + (s * BLK + H * h), H), :])
                    nc.scalar.dma_start(
                        b_sb[h][:], b_d[ds(bi2 * BLK + (s * BLK + H * h), H), :])

                # x0 = clip(x_raw) early: xb/y are free during the power
                # phase, and this keeps the post-power critical path short
                for h in range(2):
                    nc.vector.tensor_scalar(
                        xb[h], xraw_sb[h][:], 0.0, 100.0,
                        op0=mybir.AluOpType.max, op1=mybir.AluOpType.min,
                    )
                    nc.vector.tensor_copy(y_sb[h][:], xb[h])

                # ---- power iteration: v <- A^T A v (unnormalized; values
                #      stay O(20) so fp16 is safe).  v0 = ones is memset
                #      straight into the diagonal slots; nrm_k = ||v_k||^2
                #      accumulates off the critical path. ----
                for pi in range(piters):
                    for h in range(2):
                        if pi == 0:
                            nc.vector.memset(_diag_dest(y_diag[h][:]), 1.0)
                        else:
                            transpose_scatter(u_sb[h], y_diag[h][:],
                                              t1_ps_t[h], h, N)
                        _emit_matvec_half(nc, z_ps_t[h][:], y_diag[h][:], AT_b, M, h)
                    for h in range(2):
                        nc.vector.tensor_copy(av_sb[h][:], z_ps[h])
                        transpose_scatter(av_sb[h], r_diag[h][:], t2_ps_t[h], h, M)
                        _emit_matvec_half(nc, w_ps_t[h][:], r_diag[h][:], A_b, N, h)
                    for h in range(2):
                        nrm = sc_sb[h][:, pi:pi + 1]
                        nc.vector.tensor_copy(u_sb[h][:], w_ps[h])
                        nc.vector.tensor_mul(g_sb[h][:], u_sb[h][:], u_sb[h][:])
                        nc.vector.reduce_sum(nrm, g_sb[h][:],
                                             axis=mybir.AxisListType.X)

                # ---- x0 scattered (critical path: last power matvec ->
                #      transpose -> scatter -> first z); the scalar chain
                #      sigma^2 = sqrt(nrm_k/nrm_{k-1}), L = 1+2p*sigma^2,
                #      step = 1/L, beta = (sqrt(L)-1)/(sqrt(L)+1)
                #      overlaps the first FISTA matvecs on DVE/ACT. ----
                for h in range(2):
                    transpose_scatter(xb[h], y_diag[h][:], t1_ps_t[h], h, N)
                # prologue z for k=0, accumulating onto the -b preload
                for h in range(2):
                    nc.vector.tensor_scalar_mul(z_ps[h], b_sb[h][:], -1.0)
                for h in range(2):
                    _emit_matvec_half(nc, z_ps_t[h][:], y_diag[h][:], AT_b, M, h,
                                      preloaded=True)
                for h in range(2):
                    nrm1 = sc_sb[h][:, 0:1]
                    nrm2 = sc_sb[h][:, 1:2]
                    tmp = sc_sb[h][:, 2:3]
                    ratio = sc_sb[h][:, 3:4]
                    L = sc_sb[h][:, 4:5]
                    step = sc_sb[h][:, 5:6]
                    negstep = sc_sb[h][:, 6:7]
                    beta = sc_sb[h][:, 7:8]
                    sig2 = tmp   # reuse after rinv consumed
                    sqL = ratio  # reuse after sig2 consumed
                    if piters == 1:
                        # ||v0||^2 = N exactly
                        nc.vector.tensor_scalar_mul(ratio, nrm1, 1.0 / N)
                    else:
                        nprev = sc_sb[h][:, piters - 2:piters - 1]
                        ncur = sc_sb[h][:, piters - 1:piters]
                        nc.vector.reciprocal(tmp, nprev)
                        nc.vector.tensor_mul(ratio, ncur, tmp)
                    nc.scalar.sqrt(sig2, ratio)
                    nc.vector.tensor_scalar(
                        L, sig2, 2.0 * P_SLACK, 1.0,
                        op0=mybir.AluOpType.mult, op1=mybir.AluOpType.add,
                    )
                    nc.vector.reciprocal(step, L)
                    nc.vector.tensor_scalar_mul(negstep, step, -1.0)
                    nc.scalar.sqrt(sqL, L)
                    nc.vector.tensor_scalar_add(nrm1, sqL, 1.0)
                    nc.vector.reciprocal(nrm2, nrm1)
                    nc.vector.scalar_tensor_tensor(
                        beta, sqL, -1.0, nrm2,
                        op0=mybir.AluOpType.add, op1=mybir.AluOpType.mult,
                    )
                    nc.vector.tensor_scalar_mul(negbeta_c(h), beta, -1.0)
                    nc.vector.tensor_scalar_add(onepb_c(h), beta, 1.0)
                    nc.vector.tensor_scalar_mul(n2ps_c(h), step, -2.0 * P_SLACK)
                    nc.vector.tensor_scalar_mul(
                        xrs_sb[h][:], xraw_sb[h][:], 1.0 / (2.0 * P_SLACK))

                # ---- accelerated-gradient iterations (fully unrolled,
                #      software-pipelined: iteration k's z matvecs were
                #      emitted at the tail of iteration k-1, so the PE order
                #      is tr_r0 tr_r1 w0 w1 tr_y0 tr_y1 z0' z1' with the DVE
                #      update chains hidden under the matvecs.
                #      The z psum is preloaded with -b (so relu(psum) = r in
                #      one ACT op) and the w psum with (y - x_raw)/2p (so
                #      x_new = clip(y - 2p*step*psum) in one DVE op); the
                #      matvecs accumulate onto the preloads (start=False).
                #      The last iteration skips the dead y-update/scatter. ----
                for k in range(iters):
                    last = (k == iters - 1)
                    # early DVE work for this k (overlaps the in-flight z):
                    # w-psum gradient preload; mb = -beta * x_old
                    for h in range(2):
                        x_old = xb[h] if k % 2 == 0 else xa[h]
                        nc.vector.scalar_tensor_tensor(
                            w_ps[h], y_sb[h][:], 1.0 / (2.0 * P_SLACK),
                            xrs_sb[h][:],
                            op0=mybir.AluOpType.mult,
                            op1=mybir.AluOpType.subtract,
                        )
                        if not last:
                            nc.vector.tensor_scalar_mul(
                                mb_sb[h][:], x_old, negbeta_c(h))
                    # r = relu(z-psum) (already z - b) -> scatter -> w = A^T r
                    for h in range(2):
                        nc.scalar.activation(r_sb[h][:], z_ps[h],
                                             mybir.ActivationFunctionType.Relu)
                        transpose_scatter(r_sb[h], r_diag[h][:], t2_ps_t[h], h, M)
                    for h in range(2):
                        _emit_matvec_half(nc, w_ps_t[h][:], r_diag[h][:], A_b, N, h,
                                          preloaded=True)
                    # pointwise update: x_new = clip(y + n2ps*w~) then
                    # y = (1+beta)*x_new + mb
                    for h in range(2):
                        x_new = xa[h] if k % 2 == 0 else xb[h]
                        nc.vector.scalar_tensor_tensor(
                            x_new, w_ps[h], n2ps_c(h), y_sb[h][:],
                            op0=mybir.AluOpType.mult, op1=mybir.AluOpType.add,
                        )
                        nc.vector.tensor_scalar(
                            x_new, x_new, 0.0, 100.0,
                            op0=mybir.AluOpType.max, op1=mybir.AluOpType.min,
                        )
                        if not last:
                            nc.vector.scalar_tensor_tensor(
                                y_sb[h][:], x_new, onepb_c(h), mb_sb[h][:],
                                op0=mybir.AluOpType.mult, op1=mybir.AluOpType.add,
                            )
                            transpose_scatter(y_sb[h], y_diag[h][:], t1_ps_t[h], h, N)
                    if not last:
                        # z-psum preload (-b) then z matvecs for k+1
                        for h in range(2):
                            nc.vector.tensor_scalar_mul(z_ps[h], b_sb[h][:], -1.0)
                        for h in range(2):
                            _emit_matvec_half(nc, z_ps_t[h][:], y_diag[h][:],
                                              AT_b, M, h, preloaded=True)

                # final x is in the tile written by iteration iters-1
                # (iters even -> xb)
                assert iters % 2 == 0
                nc.sync.dma_start(out_d[ds(bi2 * BLK + s * BLK, BLK), :], xb_t[:])
        if rep_ctx is not None:
            rep_ctx.__exit__(None, None, None)

    if split_waits:
        _split_multiwait_insts(nc)
    return nc


_CACHED = {}


def _get_nc(**kw):
    key = ("nc",) + tuple(sorted(kw.items()))
    if key not in _CACHED:
        nc = bass.Bass("TRN2", target_bir_lowering=False, debug=False)
        build_kernel(nc, **kw)
        nc.finalize()
        _CACHED[key] = nc
    return _CACHED[key]


def _concat_in_maps(x_raw, A, b):
    per_core = []
    for c in range(N_CORES):
        sl = slice(c * B_CORE, (c + 1) * B_CORE)
        Ac = A[sl].reshape(NBLK, BLK, M, N)
        Ap = np.ascontiguousarray(
            Ac.transpose(0, 2, 1, 3)).reshape(NBLK, M, BLK * N).astype(np.float16)
        ATp = np.ascontiguousarray(
            Ac.transpose(0, 3, 1, 2)).reshape(NBLK, N, BLK * M).astype(np.float16)
        per_core.append({
            "x_raw": x_raw[sl], "Ap": Ap, "ATp": ATp, "b": b[sl],
        })
    return per_core


def timed_runs(inputs, n=5, nc=None):
    """Warm, device-resident-input executions; returns per-call wall ns."""
    import time
    import jax
    from jax.sharding import Mesh, PartitionSpec, NamedSharding
    from jax.experimental.shard_map import shard_map
    from concourse import bass2jax

    bass2jax.install_neuronx_cc_hook()
    if nc is None:
        nc = _get_nc()
    x_raw = np.ascontiguousarray(inputs["x_raw"], np.float32)
    A = np.ascontiguousarray(inputs["A"], np.float32)
    b = np.ascontiguousarray(inputs["b"], np.float32)
    per_core = _concat_in_maps(x_raw, A, b)

    in_names, out_names, out_avals = [], [], []
    for alloc in nc.m.functions[0].allocations:
        if not isinstance(alloc, mybir.MemoryLocationSet):
            continue
        name = alloc.memorylocations[0].name
        if alloc.kind == "ExternalInput":
            in_names.append(name)
        elif alloc.kind == "ExternalOutput":
            out_names.append(name)
            out_avals.append(jax.core.ShapedArray(
                tuple(alloc.tensor_shape), mybir.dt.np(alloc.dtype)))
    pid_name = nc.partition_id_tensor.name if nc.partition_id_tensor else None
    if pid_name is not None and pid_name in in_names:
        in_names.remove(pid_name)

    all_names = in_names + out_names
    if pid_name is not None:
        all_names = all_names + [pid_name]

    def _body(*args):
        operands = list(args)
        if pid_name is not None:
            operands.append(bass2jax.partition_id_tensor())
        outs = bass2jax._bass_exec_p.bind(
            *operands,
            out_avals=tuple(out_avals),
            in_names=tuple(all_names),
            out_names=tuple(out_names),
            lowering_input_output_aliases=(),
            sim_require_finite=True,
            sim_require_nnan=True,
            nc=nc,
        )
        return tuple(outs)

    devices = jax.devices()[:N_CORES]
    mesh = Mesh(np.asarray(devices), ("core",))
    nin = len(in_names) + len(out_names)
    fn = jax.jit(
        shard_map(_body, mesh=mesh, in_specs=(PartitionSpec("core"),) * nin,
                  out_specs=(PartitionSpec("core"),) * len(out_names),
                  check_rep=False),
        keep_unused=True,
    )
    sh = NamedSharding(mesh, PartitionSpec("core"))
    concat = [np.concatenate([pc[nm] for pc in per_core], axis=0) for nm in in_names]
    zeros = [np.zeros((N_CORES * av.shape[0], *av.shape[1:]), av.dtype)
             for av in out_avals]
    args = [jax.device_put(v, sh) for v in concat + zeros]
    out = fn(*args)
    jax.block_until_ready(out)  # compile + warmup
    times = []
    for _ in range(n):
        t0 = time.perf_counter()
        out = fn(*args)
        jax.block_until_ready(out)
        times.append((time.perf_counter() - t0) * 1e9)
    return times


def kernel(x_raw, A, b, lower, upper):
    x_raw = np.ascontiguousarray(x_raw, np.float32)
    A = np.ascontiguousarray(A, np.float32)
    b = np.ascontiguousarray(b, np.float32)

    nc = _get_nc()
    in_maps = _concat_in_maps(x_raw, A, b)
    res = run_bass_kernel_spmd(nc, in_maps, core_ids=list(range(N_CORES)))
    _CACHED["last_result"] = res
    out = np.concatenate([res.results[c]["x_out"] for c in range(N_CORES)], axis=0)
    return out.astype(np.float32)


# revision 34
# speedup vs baseline: 1.2167x; 1.0399x over previous
"""Trainium2 Bass kernel for batched box-constrained QP projection.

Per sample s (B=8192 total, data-parallel over 8 cores):
    min_x 0.5||x - x_raw||^2 + p*||relu(A x - b)||^2,  0 <= x <= 100

The objective is 1-strongly convex with L = 1 + 2p*sigma_max(A)^2 (~9), so
instead of the reference's 200 plain-FISTA iterations we run Nesterov's
strongly-convex accelerated projected gradient with per-sample constant
momentum beta = (sqrt(L)-1)/(sqrt(L)+1): linear convergence, 8 iterations
reach rel err ~5e-3 vs the reference (gate is 2e-2). sigma_max^2 comes from
one unnormalized power iteration via the norm-ratio estimator
sqrt(||A^TA v0||/||v0||) (estimate accuracy only perturbs the step size).

Per-core layout (1024 samples, 8 blocks of 128 = 2 halves of 64):
  - matvecs z=A y / w=A^T r run on the PE via per-sample "diagonal
    stationary" blocks in fp16 (1 cycle/row vs fp32's 4): lhsT is a [K,32]
    fp16 block that is all zeros except column (p mod 32) holding the
    sample's vector; with tile_position=(0,32*(p//32)) the result lands in
    psum row p (fp32 accumulate). 64 matmuls accumulate a [64,85] z tile.
  - all pointwise math runs batched fp32 on [64, N] tiles (DVE/ACT),
  - per half-iteration a PE transpose + one strided DVE scatter (with
    fp32->fp16 cast) rebuilds the diagonal stationaries from y / r.
  - the two halves are emitted separately (separate diag tiles and psum
    banks) so half B's matmuls overlap half A's pointwise chain on DVE/ACT.
"""
import dataclasses
import math
from contextlib import ExitStack

import numpy as np

import concourse.bass as bass
import concourse.tile as tile
from concourse import mybir
from concourse.bass import ds
from concourse.bass_utils import run_bass_kernel_spmd
from concourse.masks import make_identity

# problem constants (hardcoded per spec)
B_TOTAL = 8192
N_CORES = 8
B_CORE = B_TOTAL // N_CORES       # 1024
BLK = 128                          # samples per block
H = 64                             # samples per half
NBLK = B_CORE // BLK               # 8
N = 80                             # x dim
M = 85                             # constraint dim
P_SLACK = 1.0
ITERS = 8                          # accelerated-gradient iterations (even)
UNROLL = 4
PITERS = 1                         # power iterations
F32 = mybir.dt.float32
F16 = mybir.dt.float16


def _diag_dest(region_ap):
    """Scatter destination: for local sample p (0..63), block p occupies
    cols [32p, 32p+32); the vector goes to column offset (p mod 32).
    col = 1024*(p//32) + 33*(p%32)."""
    pstride, pcount = region_ap.ap[0]
    return dataclasses.replace(
        region_ap,
        ap=[[pstride, pcount], [1024, 2], [33, 32]],
    )


def _emit_matvec_half(nc, bank, diag_region, mov_buf, mov_cols, h, skip=True,
                      preloaded=False):
    """64 matmuls for half h: psum row 64*h+p <- <diag block p> @ mov slice.
    Col-groups 2h/2h+1 alternate per instruction so each implicit LDWEIGHTS
    overlaps the other group's in-flight MM.  With preloaded=True the psum
    region holds a bias written beforehand and every matmul accumulates."""
    for o in range(32):
        for cl in range(2):
            c = 2 * h + cl
            blk_i = 32 * c + o          # sample index within the 128-block
            lb = blk_i - 64 * h         # local sample within the half
            nc.tensor.matmul(
                bank[32 * c:32 * c + 32, 0:mov_cols],
                diag_region[:, 32 * lb:32 * lb + 32],
                mov_buf[:, mov_cols * blk_i:mov_cols * blk_i + mov_cols],
                start=(o == 0) and not preloaded, stop=(o == 31),
                tile_position=(0, 32 * c),
                skip_group_check=skip,
            )


def _split_multiwait_insts(nc):
    """walrus codegen allows only ONE sync-wait on compute/Drain instructions
    (setupSyncWait: 'Too many sync wait commands').  Tile can emit several.
    Peel all-but-one wait off onto same-engine single-wait NoOps placed just
    before the instruction (same engine + program order => identical
    semantics).  Barrier NoOps are left untouched."""
    cnt = 0
    for f in nc.m.functions:
        for b in f.blocks:
            il = list(b.instructions)
            out = []
            changed = False
            for ins in il:
                si = getattr(ins, "sync_info", None)
                if (
                    si is not None
                    and len(si.on_wait) > 1
                    and ins.opcode != "ISA"
                ):
                    waits = list(si.on_wait)
                    for j, w in enumerate(waits[:-1]):
                        nd = mybir.InstDrain(
                            name=f"{ins.name}-sw{j}", engine=ins.engine,
                            ins=[], outs=[],
                        )
                        nd.sync_info = mybir.SyncInfo(on_wait=[w], on_update=[])
                        out.append(nd)
                        cnt += 1
                    ins.sync_info = mybir.SyncInfo(
                        on_wait=[waits[-1]], on_update=list(si.on_update)
                    )
                    changed = True
                out.append(ins)
            if changed:
                b.instructions = out
    return cnt


def build_kernel(nc, split_waits=True, iters=None, piters=None, null_body=False,
                 repeat=1, no_adma=False):
    iters = ITERS if iters is None else iters
    piters = PITERS if piters is None else piters
    x_raw_d = nc.dram_tensor("x_raw", [B_CORE, N], F32, kind="ExternalInput").ap()
    A_d = nc.dram_tensor("Ap", [NBLK, M, BLK * N], F16, kind="ExternalInput").ap()
    AT_d = nc.dram_tensor("ATp", [NBLK, N, BLK * M], F16, kind="ExternalInput").ap()
    b_d = nc.dram_tensor("b", [B_CORE, M], F32, kind="ExternalInput").ap()
    out_d = nc.dram_tensor("x_out", [B_CORE, N], F32, kind="ExternalOutput").ap()

    if null_body:
        # calibration build: same external I/O, near-zero device work
        with tile.TileContext(nc) as tc, ExitStack() as ctx:
            state = ctx.enter_context(tc.tile_pool(name="state", bufs=1))
            xraw_t = state.tile([BLK, N], F32)
            with tc.For_i(0, NBLK, 1, name="blk") as bi:
                nc.sync.dma_start(xraw_t[:], x_raw_d[ds(bi * BLK, BLK), :])
                nc.vector.tensor_scalar(
                    xraw_t[:], xraw_t[:], 0.0, 100.0,
                    op0=mybir.AluOpType.max, op1=mybir.AluOpType.min,
                )
                nc.sync.dma_start(out_d[ds(bi * BLK, BLK), :], xraw_t[:])
        if split_waits:
            _split_multiwait_insts(nc)
        return nc

    with tile.TileContext(nc) as tc, ExitStack() as ctx:
        consts = ctx.enter_context(tc.tile_pool(name="consts", bufs=1))
        abuf = ctx.enter_context(tc.tile_pool(name="abuf", bufs=1))
        state = ctx.enter_context(tc.tile_pool(name="state", bufs=1))
        ps = ctx.enter_context(tc.tile_pool(name="ps", bufs=1, space="PSUM"))

        ident = consts.tile([128, 128], F32)
        make_identity(nc, ident)

        # per-half diagonal stationary regions, fp16 (off-diagonal zeros
        # persist forever)
        y_diag = [consts.tile([N, 32 * H], F16, name=f"ydiag{h}") for h in range(2)]
        r_diag = [consts.tile([M, 32 * H], F16, name=f"rdiag{h}") for h in range(2)]
        for h in range(2):
            nc.vector.memset(y_diag[h][:], 0.0)
            nc.vector.memset(r_diag[h][:], 0.0)

        # per-block A buffers, fp16 (sample-major along free dim); two sets
        # so block bi+1's DMA overlaps block bi's compute
        AT_buf = [abuf.tile([N, BLK * M], F16, name=f"ATb{s}") for s in range(2)]
        A_buf = [abuf.tile([M, BLK * N], F16, name=f"Ab{s}") for s in range(2)]

        # per-half state tiles: halves of [128, x] parents so that every
        # SB operand of a half shares the same base partition (64*h)
        def half_tiles(name, cols):
            t = state.tile([BLK, cols], F32, name=name)
            return t, [t[H * hh:H * hh + H, :] for hh in range(2)]
        y_t, y_sb = half_tiles("y_t", N)
        xa_t, xa = half_tiles("xa_t", N)
        xb_t, xb = half_tiles("xb_t", N)
        xraw_t, xraw_sb = half_tiles("xraw_t", N)
        b_t, b_sb = half_tiles("b_t", M)
        r_t, r_sb = half_tiles("r_t", M)
        g_t, g_sb = half_tiles("g_t", N)
        u_t, u_sb = half_tiles("u_t", N)
        mb_t, mb_sb = half_tiles("mb_t", N)
        av_t, av_sb = half_tiles("av_t", M)
        # scalars: 0 nrm1, 1 nrm2, 2 tmp/sig2, 3 ratio/sqL, 4 L,
        #          5 step, 6 negstep, 7 beta, 8 negbeta, 9 1+beta,
        #          10 -2p*step
        sc_t, sc_sb = half_tiles("sc_t", 11)
        negbeta_c = lambda hh: sc_sb[hh][:, 8:9]
        onepb_c = lambda hh: sc_sb[hh][:, 9:10]
        n2ps_c = lambda hh: sc_sb[hh][:, 10:11]
        # x_raw/(2p), for the w-psum gradient preload
        xrs_t, xrs_sb = half_tiles("xrs_t", N)

        # psum tiles (one bank each); half h occupies rows [64h, 64h+64)
        z_ps_t = [ps.tile([128, 512], F32, name=f"z{h}") for h in range(2)]
        w_ps_t = [ps.tile([128, 512], F32, name=f"w{h}") for h in range(2)]
        t1_ps_t = [ps.tile([128, 512], F32, name=f"t1{h}") for h in range(2)]
        t2_ps_t = [ps.tile([128, 512], F32, name=f"t2{h}") for h in range(2)]
        z_ps = [z_ps_t[hh][H * hh:H * hh + H, 0:M] for hh in range(2)]
        w_ps = [w_ps_t[hh][H * hh:H * hh + H, 0:N] for hh in range(2)]

        def scatter(dst_region, src_T):
            # src_T: psum [dim, 64] fp32; dst: fp16 diag blocks (cast on copy).
            # Runs on ACT: the next matvec blocks on this copy, and the DVE
            # always has a ready pointwise op that would delay it ~200ns.
            with tc.high_priority():
                nc.scalar.copy(
                    _diag_dest(dst_region),
                    src_T.rearrange("x (c o) -> x c o", o=32),
                )

        def transpose_scatter(vec_sb, dst_region, t_tile, half, dim):
            tp = t_tile[0:dim, 0:H]
            idh = ident[H * half:H * half + H, H * half:H * half + H]
            nc.tensor.transpose(tp, vec_sb[:, 0:dim], idh)
            scatter(dst_region, tp)

        if no_adma:
            # timing probe: load blocks 0/1 once, skip per-block A DMAs
            for s in range(2):
                nc.sync.dma_start(AT_buf[s][:], AT_d[ds(s, 1), :, :].rearrange("o n x -> (o n) x"))
                nc.sync.dma_start(A_buf[s][:], A_d[ds(s, 1), :, :].rearrange("o m x -> (o m) x"))
        rep_ctx = tc.For_i(0, repeat, 1, name="rep") if repeat > 1 else None
        if rep_ctx is not None:
            rep_ctx.__enter__()
        with tc.For_i(0, NBLK, 2, name="blk") as bi2_:
            # timing builds (repeat>1) pin DRAM addresses to block 0 so no
            # symbolic DMA APs are needed under the nested loop (SP register
            # pressure); sizes and traffic are identical.
            bi2 = 0 if repeat > 1 else bi2_
            if not no_adma:
                # set 0 is on the critical path at trip start (the enqueue
                # cannot cross the loop back-edge), so its loads are split
                # per half: the half-0 matvecs start after half the bytes
                for part in range(2):
                    nc.sync.dma_start(
                        AT_buf[0][:, part * (H * M):(part + 1) * (H * M)],
                        AT_d[ds(bi2, 1), :, part * (H * M):(part + 1) * (H * M)
                             ].rearrange("o n x -> (o n) x"))
                for part in range(2):
                    nc.sync.dma_start(
                        A_buf[0][:, part * (H * N):(part + 1) * (H * N)],
                        A_d[ds(bi2, 1), :, part * (H * N):(part + 1) * (H * N)
                            ].rearrange("o m x -> (o m) x"))
                nc.sync.dma_start(
                    AT_buf[1][:],
                    AT_d[ds(bi2 + 1, 1), :, :].rearrange("o n x -> (o n) x"))
                nc.sync.dma_start(
                    A_buf[1][:],
                    A_d[ds(bi2 + 1, 1), :, :].rearrange("o m x -> (o m) x"))
            for s in range(2):
                AT_b, A_b = AT_buf[s], A_buf[s]
                for h in range(2):
                    # small input loads go on the ACT hardware DMA queue so
                    # their waits never head-of-line-block the SP queue
                    # that streams the big A prefetches
                    nc.scalar.dma_start(
                        xraw_sb[h][:], x_raw_d[ds(bi2 * BLK + (s * BLK + H * h), H), :])
                    nc.scalar.dma_start(
                        b_sb[h][:], b_d[ds(bi2 * BLK + (s * BLK + H * h), H), :])

                # x0 = clip(x_raw) early: xb/y are free during the power
                # phase, and this keeps the post-power critical path short
                for h in range(2):
                    nc.vector.tensor_scalar(
                        xb[h], xraw_sb[h][:], 0.0, 100.0,
                        op0=mybir.AluOpType.max, op1=mybir.AluOpType.min,
                    )
                    nc.vector.tensor_copy(y_sb[h][:], xb[h])

                # ---- power iteration: v <- A^T A v (unnormalized; values
                #      stay O(20) so fp16 is safe).  v0 = ones is memset
                #      straight into the diagonal slots; nrm_k = ||v_k||^2
                #      accumulates off the critical path. ----
                for pi in range(piters):
                    for h in range(2):
                        if pi == 0:
                            nc.vector.memset(_diag_dest(y_diag[h][:]), 1.0)
                        else:
                            transpose_scatter(u_sb[h], y_diag[h][:],
                                              t1_ps_t[h], h, N)
                        _emit_matvec_half(nc, z_ps_t[h][:], y_diag[h][:], AT_b, M, h)
                    for h in range(2):
                        nc.vector.tensor_copy(av_sb[h][:], z_ps[h])
                        transpose_scatter(av_sb[h], r_diag[h][:], t2_ps_t[h], h, M)
                        _emit_matvec_half(nc, w_ps_t[h][:], r_diag[h][:], A_b, N, h)
                    for h in range(2):
                        nrm = sc_sb[h][:, pi:pi + 1]
                        nc.vector.tensor_copy(u_sb[h][:], w_ps[h])
                        nc.vector.tensor_mul(g_sb[h][:], u_sb[h][:], u_sb[h][:])
                        nc.vector.reduce_sum(nrm, g_sb[h][:],
                                             axis=mybir.AxisListType.X)

                # ---- x0 scattered (critical path: last power matvec ->
                #      transpose -> scatter -> first z); the scalar chain
                #      sigma^2 = sqrt(nrm_k/nrm_{k-1}), L = 1+2p*sigma^2,
                #      step = 1/L, beta = (sqrt(L)-1)/(sqrt(L)+1)
                #      overlaps the first FISTA matvecs on DVE/ACT. ----
                for h in range(2):
                    transpose_scatter(xb[h], y_diag[h][:], t1_ps_t[h], h, N)
                # prologue z for k=0, accumulating onto the -b preload
                for h in range(2):
                    nc.vector.tensor_scalar_mul(z_ps[h], b_sb[h][:], -1.0)
                for h in range(2):
                    _emit_matvec_half(nc, z_ps_t[h][:], y_diag[h][:], AT_b, M, h,
                                      preloaded=True)
                for h in range(2):
                    nrm1 = sc_sb[h][:, 0:1]
                    nrm2 = sc_sb[h][:, 1:2]
                    tmp = sc_sb[h][:, 2:3]
                    ratio = sc_sb[h][:, 3:4]
                    L = sc_sb[h][:, 4:5]
                    step = sc_sb[h][:, 5:6]
                    negstep = sc_sb[h][:, 6:7]
                    beta = sc_sb[h][:, 7:8]
                    sig2 = tmp   # reuse after rinv consumed
                    sqL = ratio  # reuse after sig2 consumed
                    if piters == 1:
                        # ||v0||^2 = N exactly
                        nc.vector.tensor_scalar_mul(ratio, nrm1, 1.0 / N)
                    else:
                        nprev = sc_sb[h][:, piters - 2:piters - 1]
                        ncur = sc_sb[h][:, piters - 1:piters]
                        nc.vector.reciprocal(tmp, nprev)
                        nc.vector.tensor_mul(ratio, ncur, tmp)
                    nc.scalar.sqrt(sig2, ratio)
                    nc.vector.tensor_scalar(
                        L, sig2, 2.0 * P_SLACK, 1.0,
                        op0=mybir.AluOpType.mult, op1=mybir.AluOpType.add,
                    )
                    nc.vector.reciprocal(step, L)
                    nc.vector.tensor_scalar_mul(negstep, step, -1.0)
                    nc.scalar.sqrt(sqL, L)
                    nc.vector.tensor_scalar_add(nrm1, sqL, 1.0)
                    nc.vector.reciprocal(nrm2, nrm1)
                    nc.vector.scalar_tensor_tensor(
                        beta, sqL, -1.0, nrm2,
                        op0=mybir.AluOpType.add, op1=mybir.AluOpType.mult,
                    )
                    nc.vector.tensor_scalar_mul(negbeta_c(h), beta, -1.0)
                    nc.vector.tensor_scalar_add(onepb_c(h), beta, 1.0)
                    nc.vector.tensor_scalar_mul(n2ps_c(h), step, -2.0 * P_SLACK)
                    nc.vector.tensor_scalar_mul(
                        xrs_sb[h][:], xraw_sb[h][:], 1.0 / (2.0 * P_SLACK))

                # ---- accelerated-gradient iterations (fully unrolled,
                #      software-pipelined: iteration k's z matvecs were
                #      emitted at the tail of iteration k-1, so the PE order
                #      is tr_r0 tr_r1 w0 w1 tr_y0 tr_y1 z0' z1' with the DVE
                #      update chains hidden under the matvecs.
                #      The z psum is preloaded with -b (so relu(psum) = r in
                #      one ACT op) and the w psum with (y - x_raw)/2p (so
                #      x_new = clip(y - 2p*step*psum) in one DVE op); the
                #      matvecs accumulate onto the preloads (start=False).
                #      The last iteration skips the dead y-update/scatter. ----
                for k in range(iters):
                    last = (k == iters - 1)
                    # early DVE work for this k (overlaps the in-flight z):
                    # w-psum gradient preload; mb = -beta * x_old
                    for h in range(2):
                        x_old = xb[h] if k % 2 == 0 else xa[h]
                        nc.vector.scalar_tensor_tensor(
                            w_ps[h], y_sb[h][:], 1.0 / (2.0 * P_SLACK),
                            xrs_sb[h][:],
                            op0=mybir.AluOpType.mult,
                            op1=mybir.AluOpType.subtract,
                        )
                        if not last:
                            nc.vector.tensor_scalar_mul(
                                mb_sb[h][:], x_old, negbeta_c(h))
                    # r = relu(z-psum) (already z - b) -> scatter -> w = A^T r
                    for h in range(2):
                        nc.scalar.activation(r_sb[h][:], z_ps[h],
                                             mybir.ActivationFunctionType.Relu)
                        transpose_scatter(r_sb[h], r_diag[h][:], t2_ps_t[h], h, M)
                    for h in range(2):
                        _emit_matvec_half(nc, w_ps_t[h][:], r_diag[h][:], A_b, N, h,
                                          preloaded=True)
                    # pointwise update: x_new = clip(y + n2ps*w~) then
                    # y = (1+beta)*x_new + mb
                    for h in range(2):
                        x_new = xa[h] if k % 2 == 0 else xb[h]
                        nc.vector.scalar_tensor_tensor(
                            x_new, w_ps[h], n2ps_c(h), y_sb[h][:],
                            op0=mybir.AluOpType.mult, op1=mybir.AluOpType.add,
                        )
                        nc.vector.tensor_scalar(
                            x_new, x_new, 0.0, 100.0,
                            op0=mybir.AluOpType.max, op1=mybir.AluOpType.min,
                        )
                        if not last:
                            nc.vector.scalar_tensor_tensor(
                                y_sb[h][:], x_new, onepb_c(h), mb_sb[h][:],
                                op0=mybir.AluOpType.mult, op1=mybir.AluOpType.add,
                            )
                            transpose_scatter(y_sb[h], y_diag[h][:], t1_ps_t[h], h, N)
                    if not last:
                        # z-psum preload (-b) then z matvecs for k+1
                        for h in range(2):
                            nc.vector.tensor_scalar_mul(z_ps[h], b_sb[h][:], -1.0)
                        for h in range(2):
                            _emit_matvec_half(nc, z_ps_t[h][:], y_diag[h][:],
                                              AT_b, M, h, preloaded=True)

                # final x is in the tile written by iteration iters-1
                # (iters even -> xb)
                assert iters % 2 == 0
                nc.sync.dma_start(out_d[ds(bi2 * BLK + s * BLK, BLK), :], xb_t[:])
        if rep_ctx is not None:
            rep_ctx.__exit__(None, None, None)

    if split_waits:
        _split_multiwait_insts(nc)
    return nc


_CACHED = {}


def _get_nc(**kw):
    key = ("nc",) + tuple(sorted(kw.items()))
    if key not in _CACHED:
        nc = bass.Bass("TRN2", target_bir_lowering=False, debug=False)
        build_kernel(nc, **kw)
        nc.finalize()
        _CACHED[key] = nc
    return _CACHED[key]


def _concat_in_maps(x_raw, A, b):
    per_core = []
    for c in range(N_CORES):
        sl = slice(c * B_CORE, (c + 1) * B_CORE)
        Ac = A[sl].reshape(NBLK, BLK, M, N)
        Ap = np.ascontiguousarray(
            Ac.transpose(0, 2, 1, 3)).reshape(NBLK, M, BLK * N).astype(np.float16)
        ATp = np.ascontiguousarray(
            Ac.transpose(0, 3, 1, 2)).reshape(NBLK, N, BLK * M).astype(np.float16)
        per_core.append({
            "x_raw": x_raw[sl], "Ap": Ap, "ATp": ATp, "b": b[sl],
        })
    return per_core


def timed_runs(inputs, n=5, nc=None):
    """Warm, device-resident-input executions; returns per-call wall ns."""
    import time
    import jax
    from jax.sharding import Mesh, PartitionSpec, NamedSharding
    from jax.experimental.shard_map import shard_map
    from concourse import bass2jax

    bass2jax.install_neuronx_cc_hook()
    if nc is None:
        nc = _get_nc()
    x_raw = np.ascontiguousarray(inputs["x_raw"], np.float32)
    A = np.ascontiguousarray(inputs["A"], np.float32)
    b = np.ascontiguousarray(inputs["b"], np.float32)
    per_core = _concat_in_maps(x_raw, A, b)

    in_names, out_names, out_avals = [], [], []
    for alloc in nc.m.functions[0].allocations:
        if not isinstance(alloc, mybir.MemoryLocationSet):
            continue
        name = alloc.memorylocations[0].name
        if alloc.kind == "ExternalInput":
            in_names.append(name)
        elif alloc.kind == "ExternalOutput":
            out_names.append(name)
            out_avals.append(jax.core.ShapedArray(
                tuple(alloc.tensor_shape), mybir.dt.np(alloc.dtype)))
    pid_name = nc.partition_id_tensor.name if nc.partition_id_tensor else None
    if pid_name is not None and pid_name in in_names:
        in_names.remove(pid_name)

    all_names = in_names + out_names
    if pid_name is not None:
        all_names = all_names + [pid_name]

    def _body(*args):
        operands = list(args)
        if pid_name is not None:
            operands.append(bass2jax.partition_id_tensor())
        outs = bass2jax._bass_exec_p.bind(
            *operands,
            out_avals=tuple(out_avals),
            in_names=tuple(all_names),
            out_names=tuple(out_names),
            lowering_input_output_aliases=(),
            sim_require_finite=True,
            sim_require_nnan=True,
            nc=nc,
        )
        return tuple(outs)

    devices = jax.devices()[:N_CORES]
    mesh = Mesh(np.asarray(devices), ("core",))
    nin = len(in_names) + len(out_names)
    fn = jax.jit(
        shard_map(_body, mesh=mesh, in_specs=(PartitionSpec("core"),) * nin,
                  out_specs=(PartitionSpec("core"),) * len(out_names),
                  check_rep=False),
        keep_unused=True,
    )
    sh = NamedSharding(mesh, PartitionSpec("core"))
    concat = [np.concatenate([pc[nm] for pc in per_core], axis=0) for nm in in_names]
    zeros = [np.zeros((N_CORES * av.shape[0], *av.shape[1:]), av.dtype)
             for av in out_avals]
    args = [jax.device_put(v, sh) for v in concat + zeros]
    out = fn(*args)
    jax.block_until_ready(out)  # compile + warmup
    times = []
    for _ in range(n):
        t0 = time.perf_counter()
        out = fn(*args)
        jax.block_until_ready(out)
        times.append((time.perf_counter() - t0) * 1e9)
    return times


def kernel(x_raw, A, b, lower, upper):
    x_raw = np.ascontiguousarray(x_raw, np.float32)
    A = np.ascontiguousarray(A, np.float32)
    b = np.ascontiguousarray(b, np.float32)

    nc = _get_nc()
    in_maps = _concat_in_maps(x_raw, A, b)
    res = run_bass_kernel_spmd(nc, in_maps, core_ids=list(range(N_CORES)))
    _CACHED["last_result"] = res
    out = np.concatenate([res.results[c]["x_out"] for c in range(N_CORES)], axis=0)
    return out.astype(np.float32)
